# revision 15
# baseline (speedup 1.0000x reference)
"""DuQuant-style W4A4 fake-quantized linear layer on 8 Trainium2 NeuronCores.

Math (validated against the reference on host):
  reference: out = fq(x) @ fq(w).T + bias, where fq rotates by block-diagonal
  R, quantizes asymmetrically to 4 bits per row over the full 4096 features,
  dequantizes, and de-rotates.

  Because R is orthogonal, the two de-rotations cancel inside the matmul:
      (Xdq Br)(Wdq Br).T = Xdq Wdq.T,   Br = blockdiag(R.T)
  and because min <= 0 <= max (forced), the zero-point cancels exactly:
      (clip(round(xr/s)+zp,0,15)-zp)*s = round(xr/s)*s
  so each operand is an integer in [-15, 15] times a per-row scale.  The
  integers are exact in fp8e4m3, making the main 275-GFLOP matmul EXACT in
  fp8 (DoubleRow pairs also stay exact: e6m3 holds ints to 15, e10m10 holds
  products to 225); the scales are applied to the fp32 accumulator after.

Sharding: tokens 8-way (x-side quant fully core-local).  Weight quant is
split 8-way by out-row block; each core quantizes+transposes its 512 rows
and the fp8 results are AllGather'd on-device, overlapped with x-quant.

Rotation precision: 3-term bf16 split (x_hi@R_hi + x_lo@R_hi + x_hi@R_lo).

v2 structure (per core):
  ctx1: consts + w-quant (4 stripes)
  AllGather (w codes + w scales) issued, NOT waited
  ctx2: x-quant (8 stripes, software-pipelined); sync queue waits the AG
        then prefetches bias/sw broadcast tiles
  ctx3: main matmul, g-outer, DoubleRow fp8, fused epilogue:
        DVE scalar_tensor_tensor (psum*sx)*sw -> bf16, GpSimd +bias,
        SWDGE DMA with bf16->fp32 cast to DRAM.
"""
import numpy as np

import concourse.bass as bass
import concourse.tile as tile
from concourse import mybir
from concourse.bass_utils import run_bass_kernel_spmd
from concourse.masks import make_identity
from concourse.vector_clock import ScopedClock
from contextlib import ExitStack

N_CORES = 8
TOK = 8192          # total tokens (4*2048)
F = 4096            # features (in and out)
TPC = TOK // N_CORES   # tokens per core = 1024
WPC = F // N_CORES     # weight rows per core = 512
NB = F // 128          # rotation blocks = 32

f32 = mybir.dt.float32
bf16 = mybir.dt.bfloat16
fp8 = mybir.dt.float8e4
AF = mybir.ActivationFunctionType
ALU = mybir.AluOpType
DR = mybir.MatmulPerfMode.DoubleRow

MAGIC = float(np.float32(1.5 * 2 ** 23))
INV15 = float(np.float32(1.0) / np.float32(15.0))

# ---------------------------------------------------------------------------
# Workaround: this container's walrus rejects instructions with more than one
# embedded sync-wait.  Patch the Tile tail drain and post-split all waits.
# ---------------------------------------------------------------------------
_split_counter = [0]


def _patched_drain_and_barrier(self, tick_clock, wait_clock):
    nc = self.nc
    collector = nc.sync.nop(nofuse=True)
    wait_clock.add_sem_waits(collector.ins, ScopedClock({None: tick_clock.global_clock}))
    si = collector.ins.sync_info
    waits = list(si.on_wait) if si is not None else []
    updates = list(si.on_update) if si is not None else []
    collector.ins.sync_info = mybir.SyncInfo(on_wait=waits[:1], on_update=updates)
    for w in waits[1:]:
        n = nc.sync.nop(nofuse=True)
        n.ins.sync_info = mybir.SyncInfo(on_wait=[w], on_update=[])
    nc.sync.drain()
    nc.all_engine_barrier()
    assert self.sems is not None
    popped = nc._tile_sem_poison_stack.pop()
    assert popped is self._sem_poison
    nc.clear_and_free_semaphores(list(self.sems.allocated().values()))
    nc.all_engine_barrier()


tile.TileContext._drain_and_barrier = _patched_drain_and_barrier


def _split_waits(nc, max_waits=1):
    for fn in nc.m.functions:
        for bb in fn.blocks:
            insts = bb.instructions
            out = []
            changed = False
            for inst in insts:
                si = inst.sync_info
                waits = list(si.on_wait) if si is not None else []
                if len(waits) > max_waits:
                    keep = waits[-max_waits:]
                    extra = waits[:-max_waits]
                    for i in range(0, len(extra), max_waits):
                        _split_counter[0] += 1
                        n = mybir.InstNoOp(name=f"I-wsplit-{_split_counter[0]}", ins=[], outs=[])
                        n.engine = inst.engine
                        n.sync_info = mybir.SyncInfo(on_wait=extra[i:i + max_waits], on_update=[])
                        nc.register_instruction(n, overwrite=True)
                        out.append(n)
                    inst.sync_info = mybir.SyncInfo(
                        on_wait=keep, on_update=list(si.on_update) if si is not None else [])
                    changed = True
                out.append(inst)
            if changed:
                bb.instructions = out


# ---------------------------------------------------------------------------
# Quant-side stripe pipeline (shared by w and x)
# ---------------------------------------------------------------------------

class _QuantPools:
    def __init__(self, nc, tc, ctx, sfx=""):
        self.nc = nc
        self.sb = ctx.enter_context(tc.tile_pool(name="q_sb" + sfx, bufs=2))
        self.sb_small = ctx.enter_context(tc.tile_pool(name="q_sbs" + sfx, bufs=2))
        self.ps_t = ctx.enter_context(tc.tile_pool(name="q_pst" + sfx, bufs=3, space="PSUM"))
        self.ps_r = ctx.enter_context(tc.tile_pool(name="q_psr" + sfx, bufs=3, space="PSUM"))
        self.ps_t2 = ctx.enter_context(tc.tile_pool(name="q_ps2" + sfx, bufs=2, space="PSUM"))


def _quant_stripes(nc, pools, ident, ident_bf, Rhi, Rlo, zeros, stripes):
    """Fake-quantize a sequence of [128, 4096] stripes.

    stripes: list of (src_dram_slice, dstT, dst_scale_col_ap, s_idx) where
    dstT is [128, NB, n*128] fp8 (codes transposed) and dst_scale_col_ap is
    the [128, 1] slice to write the per-row scale into.

    Software-pipelined: stage B (round+transpose-out) of stripe s-1 is
    emitted after stage A (in+transpose+rotate+minmax) of stripe s so the
    PE/engine queues never stall on the scale chain.
    """
    pend = None  # (qb_tile, dstT, col)

    def stage_b(qb, dstT, col):
        # pass-2: transpose bf16 codes, drain to fp8 into dstT
        for bg in range(NB // 4):
            pt2 = pools.ps_t2.tile([128, 4, 128], bf16, tag="pt2")
            for bb in range(4):
                b = bg * 4 + bb
                nc.tensor.transpose(pt2[:, bb, :], qb[:, 128 * b:128 * (b + 1)],
                                    ident_bf[:])
            dv = dstT[:, 4 * bg:4 * (bg + 1), 128 * col:128 * (col + 1)]
            nc.scalar.activation(dv, pt2[:], AF.Copy)

    for (src, dstT, scale_col, col) in stripes:
        xs = pools.sb.tile([128, F], f32, tag="xs")
        nc.gpsimd.dma_start(out=xs[:], in_=src)

        # pass-1: transpose raw x, split hi/lo straight off the psum banks
        hiT = pools.sb.tile([128, F], bf16, tag="hiT")
        loT = pools.sb.tile([128, F], bf16, tag="loT")
        for bg in range(NB // 4):
            pt = pools.ps_t.tile([128, 512], f32, tag="pt")
            for bb in range(4):
                b = bg * 4 + bb
                nc.tensor.transpose(pt[:, 128 * bb:128 * (bb + 1)],
                                    xs[:, 128 * b:128 * (b + 1)], ident[:])
            hv = hiT[:, 512 * bg:512 * (bg + 1)]
            nc.scalar.activation(hv, pt[:], AF.Copy)
            nc.vector.tensor_tensor(out=loT[:, 512 * bg:512 * (bg + 1)],
                                    in0=pt[:], in1=hv, op=ALU.subtract)

        # rotate 3-term into psum; drain + per-bank min/max partials
        xr = pools.sb.tile([128, F], f32, tag="xr")
        mnp = pools.sb_small.tile([128, 8], f32, tag="mnp")
        mxp = pools.sb_small.tile([128, 8], f32, tag="mxp")
        for bg in range(NB // 4):
            pr = pools.ps_r.tile([128, 512], f32, tag="pr")
            for bb in range(4):
                b = bg * 4 + bb
                sl = pr[:, 128 * bb:128 * (bb + 1)]
                h = hiT[:, 128 * b:128 * (b + 1)]
                l = loT[:, 128 * b:128 * (b + 1)]
                nc.tensor.matmul(sl, h, Rhi[:], start=True, stop=False)
                nc.tensor.matmul(sl, h, Rlo[:], start=False, stop=False)
                nc.tensor.matmul(sl, l, Rhi[:], start=False, stop=True)
            nc.vector.tensor_reduce(out=mnp[:, bg:bg + 1], in_=pr[:],
                                    axis=mybir.AxisListType.X, op=ALU.min)
            nc.vector.tensor_reduce(out=mxp[:, bg:bg + 1], in_=pr[:],
                                    axis=mybir.AxisListType.X, op=ALU.max)
            nc.scalar.activation(xr[:, 512 * bg:512 * (bg + 1)], pr[:], AF.Copy)

        # scale = max((max(mx,0) - min(mn,0)) * (1/15), 1e-5); inv = 1/scale
        mn = pools.sb_small.tile([128, 1], f32, tag="mn")
        mx = pools.sb_small.tile([128, 1], f32, tag="mx")
        nc.vector.tensor_reduce(out=mn[:], in_=mnp[:], axis=mybir.AxisListType.X, op=ALU.min)
        nc.vector.tensor_reduce(out=mx[:], in_=mxp[:], axis=mybir.AxisListType.X, op=ALU.max)
        nc.vector.tensor_scalar(out=mn[:], in0=mn[:], scalar1=0.0, scalar2=None, op0=ALU.min)
        nc.vector.tensor_scalar(out=mx[:], in0=mx[:], scalar1=0.0, scalar2=None, op0=ALU.max)
        rng = pools.sb_small.tile([128, 1], f32, tag="rng")
        nc.vector.tensor_tensor(out=rng[:], in0=mx[:], in1=mn[:], op=ALU.subtract)
        nc.vector.tensor_scalar(out=scale_col, in0=rng[:], scalar1=INV15, scalar2=1e-5,
                                op0=ALU.mult, op1=ALU.max)
        inv = pools.sb_small.tile([128, 1], f32, tag="inv")
        nc.vector.reciprocal(inv[:], scale_col)

        # integer codes: qm = xr*inv + MAGIC (RNE, scalar), qb = qm - MAGIC (bf16)
        qb = pools.sb.tile([128, F], bf16, tag="qb")
        nc.scalar.activation(xr[:], xr[:], AF.Copy, bias=MAGIC, scale=inv[:])
        nc.vector.tensor_scalar(out=qb[:], in0=xr[:], scalar1=MAGIC, scalar2=None,
                                op0=ALU.subtract)

        # emit previous stripe's stage B now (keeps PE fed during scale chain)
        if pend is not None:
            stage_b(*pend)
        pend = (qb, dstT, col)

    stage_b(*pend)


# ---------------------------------------------------------------------------
# Device program
# ---------------------------------------------------------------------------

def build_program(nrep=1):
    nc = bass.Bass("TRN2", target_bir_lowering=False, debug=False, num_devices=N_CORES)
    core_ids = list(range(N_CORES))

    x_d = nc.dram_tensor("x", [TPC, F], f32, kind="ExternalInput").ap()
    w_d = nc.dram_tensor("w", [WPC, F], f32, kind="ExternalInput").ap()
    bias_d = nc.dram_tensor("bias", [1, F], f32, kind="ExternalInput").ap()
    R_d = nc.dram_tensor("R", [128, 128], f32, kind="ExternalInput").ap()
    out_d = nc.dram_tensor("out", [TPC, F], f32, kind="ExternalOutput").ap()

    contrib_w = nc.dram_tensor("contrib_w", [F, WPC], fp8)
    gathered_w = nc.dram_tensor("gathered_w", [N_CORES * F, WPC], fp8, addr_space="Shared")
    contrib_sw = nc.dram_tensor("contrib_sw", [WPC // 128, 128], f32)
    gathered_sw = nc.dram_tensor("gathered_sw", [N_CORES * (WPC // 128), 128], f32,
                                 addr_space="Shared")

    # static SBUF tensors that survive across TileContexts
    xqT = nc.alloc_sbuf_tensor("xqT_st", [128, NB, TPC], fp8).ap()
    sx_st = nc.alloc_sbuf_tensor("sx_st", [128, TPC // 128], f32).ap()

    for rep in range(nrep):
        sfx = f"_r{rep}" if rep else ""

        # ---------------- ctx1: consts + w quant ----------------
        with tile.TileContext(nc) as tc, ExitStack() as ctx:
            const = ctx.enter_context(tc.tile_pool(name="const" + sfx, bufs=1))
            ident = const.tile([128, 128], f32)
            make_identity(nc, ident)
            ident_bf = const.tile([128, 128], bf16)
            make_identity(nc, ident_bf)
            Rs = const.tile([128, 128], f32)
            nc.gpsimd.dma_start(out=Rs[:], in_=R_d[:])
            Rhi = const.tile([128, 128], bf16)
            nc.vector.tensor_copy(Rhi[:], Rs[:])
            Rlo = const.tile([128, 128], bf16)
            nc.vector.tensor_tensor(out=Rlo[:], in0=Rs[:], in1=Rhi[:], op=ALU.subtract)
            zeros = const.tile([128, 512], f32)
            nc.gpsimd.memset(zeros[:], 0.0)
            wq_pool = ctx.enter_context(tc.tile_pool(name="wqT_sb" + sfx, bufs=1))
            wqT = wq_pool.tile([128, NB, WPC], fp8)
            sw_pool = ctx.enter_context(tc.tile_pool(name="sw_sb" + sfx, bufs=1))
            sw_t = sw_pool.tile([128, WPC // 128], f32)

            pools = _QuantPools(nc, tc, ctx, sfx="w" + sfx)
            _quant_stripes(
                nc, pools, ident, ident_bf, Rhi, Rlo, zeros,
                [(w_d[128 * s:128 * (s + 1), :], wqT, sw_t[:, s:s + 1], s)
                 for s in range(WPC // 128)])

            # ship wqT + sw to DRAM for the gather
            nc.gpsimd.dma_start(
                out=contrib_w.ap().rearrange("(b p) r -> p b r", p=128), in_=wqT[:])
            nc.gpsimd.dma_start(
                out=contrib_sw.ap().rearrange("s p -> p s"), in_=sw_t[:])

        with nc.semaphore("cc_sem" + sfx) as cc_sem:
            nc.gpsimd.collective_compute(
                "AllGather", ALU.bypass, replica_groups=[core_ids],
                ins=[contrib_w[:]], outs=[gathered_w[:]],
            ).then_inc(cc_sem)
            nc.gpsimd.collective_compute(
                "AllGather", ALU.bypass, replica_groups=[core_ids],
                ins=[contrib_sw[:]], outs=[gathered_sw[:]],
            ).then_inc(cc_sem)

            # ---------------- ctx2: x quant (AG runs concurrently) --------
            with tile.TileContext(nc) as tc, ExitStack() as ctx:
                const = ctx.enter_context(tc.tile_pool(name="c2" + sfx, bufs=1))
                ident = const.tile([128, 128], f32)
                make_identity(nc, ident)
                ident_bf = const.tile([128, 128], bf16)
                make_identity(nc, ident_bf)
                Rs = const.tile([128, 128], f32)
                nc.gpsimd.dma_start(out=Rs[:], in_=R_d[:])
                Rhi = const.tile([128, 128], bf16)
                nc.vector.tensor_copy(Rhi[:], Rs[:])
                Rlo = const.tile([128, 128], bf16)
                nc.vector.tensor_tensor(out=Rlo[:], in0=Rs[:], in1=Rhi[:], op=ALU.subtract)
                zeros = const.tile([128, 512], f32)
                nc.gpsimd.memset(zeros[:], 0.0)
                pools = _QuantPools(nc, tc, ctx, sfx="x" + sfx)
                _quant_stripes(
                    nc, pools, ident, ident_bf, Rhi, Rlo, zeros,
                    [(x_d[128 * s:128 * (s + 1), :], xqT, sx_st[:, s:s + 1], s)
                     for s in range(TPC // 128)])

            # order everything after ctx2 behind the AllGather
            nc.gpsimd.wait_ge(cc_sem, 2)

            # ---------------- ctx3: main matmul ----------------
            with tile.TileContext(nc) as tc, ExitStack() as ctx:
                c3 = ctx.enter_context(tc.tile_pool(name="mm_c3" + sfx, bufs=1))
                biasb = c3.tile([128, F], f32)
                nc.sync.dma_start(out=biasb[:], in_=bias_d[:].partition_broadcast(128))
                swb = c3.tile([128, F], f32)
                nc.sync.dma_start(
                    out=swb[:],
                    in_=gathered_sw.ap().rearrange("(o s) p -> o (s p)", o=1)
                    .partition_broadcast(128))

                wq_sb = ctx.enter_context(tc.tile_pool(name="mm_wq" + sfx, bufs=3))
                e_sb = ctx.enter_context(tc.tile_pool(name="mm_e" + sfx, bufs=6))
                pso = ctx.enter_context(tc.tile_pool(name="mm_ps" + sfx, bufs=8, space="PSUM"))

                for g in range(N_CORES):
                    wq_t = wq_sb.tile([128, NB, WPC], fp8, tag="wq_t")
                    nc.sync.dma_start(
                        out=wq_t[:],
                        in_=gathered_w[F * g:F * (g + 1), :].rearrange(
                            "(b p) r -> p b r", p=128))
                    for tt in range(TPC // 128):
                        po = pso.tile([128, WPC], f32, tag="po")
                        for kk in range(NB // 2):
                            nc.tensor.matmul(
                                po[:],
                                xqT[:, 2 * kk:2 * kk + 2, 128 * tt:128 * (tt + 1)],
                                wq_t[:, 2 * kk:2 * kk + 2, :],
                                start=(kk == 0), stop=(kk == NB // 2 - 1),
                                perf_mode=DR)
                        # epilogue: e2 = (po * sx) * sw  (DVE, reads PSUM)
                        e2 = e_sb.tile([128, WPC], bf16, tag="e2")
                        nc.vector.scalar_tensor_tensor(
                            out=e2[:], in0=po[:], scalar=sx_st[:, tt:tt + 1],
                            in1=swb[:, WPC * g:WPC * (g + 1)],
                            op0=ALU.mult, op1=ALU.mult)
                        # e3 = e2 + bias  (GpSimd), then DMA out with cast
                        e3 = e_sb.tile([128, WPC], bf16, tag="e3")
                        nc.gpsimd.tensor_tensor(
                            out=e3[:], in0=e2[:],
                            in1=biasb[:, WPC * g:WPC * (g + 1)], op=ALU.add)
                        nc.gpsimd.dma_start(
                            out=out_d[128 * tt:128 * (tt + 1), WPC * g:WPC * (g + 1)],
                            in_=e3[:])

    _split_waits(nc, max_waits=1)
    # tensor_tensor_reduce lowers to an extended-inst InstISA subclass whose
    # .instr bytes are only populated by this pass (else "ISA wrong length")
    from concourse.library_overlay import lower_extended_insts
    lower_extended_insts(nc)
    return nc


_PROGRAM = None


def _get_program():
    global _PROGRAM
    if _PROGRAM is None:
        _PROGRAM = build_program()
    return _PROGRAM


def kernel(input, weight, bias, R):
    input = np.ascontiguousarray(np.asarray(input, dtype=np.float32))
    weight = np.ascontiguousarray(np.asarray(weight, dtype=np.float32))
    bias = np.ascontiguousarray(np.asarray(bias, dtype=np.float32))
    R = np.ascontiguousarray(np.asarray(R, dtype=np.float32))

    B, S, F_ = input.shape
    x_flat = input.reshape(B * S, F_)

    nc = _get_program()
    in_maps = []
    for c in range(N_CORES):
        in_maps.append({
            "x": x_flat[TPC * c:TPC * (c + 1)],
            "w": weight[WPC * c:WPC * (c + 1)],
            "bias": bias.reshape(1, F_),
            "R": R,
        })
    res = run_bass_kernel_spmd(nc, in_maps, list(range(N_CORES))).results
    out = np.concatenate([res[c]["out"] for c in range(N_CORES)], axis=0)
    return out.reshape(B, S, F_)
